# revision 7
# baseline (speedup 1.0000x reference)
"""Trainium2 Bass kernel for dictionary-matching KNN (MR fingerprinting style).

Math (validated against the reference numerically):
  ref computes, per pixel p (N = 192*192 = 36864) and dictionary atom d (D = 4000):
      dist(d, p) = || normalize(sig_masked)[p] - normalize(db_masked)[d] ||
  and returns argmin_d dist, min_d dist, plus t2/b1 table lookups of the argmin.

  Because both vectors are unit-norm over the 8 SE columns (mask keeps even echoes),
      argmin_d dist == argmax_d  <x_masked[p], db_hat[d]>
  where x_masked is the *unnormalized* masked signal (per-pixel positive scale does
  not change the argmax) and db_hat is the L2-normalized masked dictionary.
  min_dist is recovered as sqrt(max(2 - 2*m/n, 0)) with m the max dot product and
  n = ||x_masked[p]||.

Device work per core (1/8 of the pixels): normalize dictionary (replicated),
4000x4608 fp32 matmul on PE, per-pixel max via TENSOR_TENSOR_REDUCE, exact index
via MAX_INDEX (first-occurrence semantics matches jnp.argmin tie-break), min_dist
via sqrt with a Newton refinement. Host does only layout work (shard/transpose/
concat) plus the final t2/b1 table gathers from device-computed indices.
"""

import os
import sys

import numpy as np

for _p in ("/opt/trn_rl_repo", "/root/.axon_site/_ro/trn_rl_repo"):
    if os.path.isdir(_p) and _p not in sys.path:
        sys.path.insert(0, _p)

import concourse.bacc as bacc
import concourse.bass as bass
import concourse.mybir as mybir
import concourse.tile as tile
from concourse import bass_utils

F32 = mybir.dt.float32
U32 = mybir.dt.uint32
Alu = mybir.AluOpType
Act = mybir.ActivationFunctionType
AX = mybir.AxisListType

# Problem constants (hardcoded per contract).
NX, NY, ETL = 192, 192, 16
N = NX * NY            # 36864 pixels
D = 4000               # dictionary atoms
NCORES = 8
NPC = N // NCORES      # 4608 pixels per core
P = 128                # pixels per tile (partitions)
T = NPC // P           # 36 tiles per core
HALF = 2000            # atoms per PSUM half
CHUNKS = ((0, 512), (512, 512), (1024, 512), (1536, 464))  # matmul N-chunks per half
DP = 125               # atoms per partition-chunk in dictionary prep (4000 = 32*125)
DC = 32                # dictionary partition-chunks


def _rsqrt_refined(nc, pool, ss, n_iter=2, name="rs"):
    """1/sqrt(ss) with Newton refinement (ACT Sqrt table can be sloppy)."""
    pp, fr = ss.shape[0], ss.shape[1]
    st = pool.tile([pp, fr], F32, name=f"{name}_st")
    nc.scalar.activation(st[:], ss, Act.Sqrt)
    r = pool.tile([pp, fr], F32, name=f"{name}_r0")
    nc.vector.reciprocal(r[:], st[:])
    for i in range(n_iter):
        a = pool.tile([pp, fr], F32, name=f"{name}_a{i}")
        nc.vector.tensor_mul(a[:], r[:], r[:])
        b = pool.tile([pp, fr], F32, name=f"{name}_b{i}")
        nc.vector.tensor_mul(b[:], a[:], ss)
        c = pool.tile([pp, fr], F32, name=f"{name}_c{i}")
        nc.vector.tensor_scalar(c[:], b[:], -0.5, 1.5, op0=Alu.mult, op1=Alu.add)
        r2 = pool.tile([pp, fr], F32, name=f"{name}_r{i + 1}")
        nc.vector.tensor_mul(r2[:], r[:], c[:])
        r = r2
    return r


def build_program(num_devices=NCORES):
    nc = bacc.Bacc(
        "TRN2", target_bir_lowering=False, debug=False, num_devices=num_devices
    )

    xT8_d = nc.dram_tensor("xT8", [8, NPC], F32, kind="ExternalInput")
    xin_d = nc.dram_tensor("xin", [NPC, ETL], F32, kind="ExternalInput")
    dbm_d = nc.dram_tensor("dbm", [D, ETL], F32, kind="ExternalInput")
    idn_d = nc.dram_tensor("idn", [DP, DP], F32, kind="ExternalInput")
    oidx_d = nc.dram_tensor("oidx", [P, T], U32, kind="ExternalOutput")
    omd_d = nc.dram_tensor("omd", [P, T], F32, kind="ExternalOutput")
    om_d = nc.dram_tensor("om", [P, T], F32, kind="ExternalOutput")

    with tile.TileContext(nc) as tc:
        with (
            tc.tile_pool(name="sbp", bufs=1) as sbp,
            tc.tile_pool(name="psp", bufs=2, space="PSUM") as psp,
            tc.tile_pool(name="esp", bufs=2) as esp,
            tc.tile_pool(name="scp", bufs=2) as scp,
            tc.tile_pool(name="mxp", bufs=2) as mxp,
        ):
            # ---------------- prologue: loads ----------------
            xT8 = sbp.tile([8, NPC], F32, name="xT8s")
            nc.sync.dma_start(xT8[:], xT8_d.ap())
            xin = sbp.tile([P, T * ETL], F32, name="xins")
            nc.sync.dma_start(
                xin[:].rearrange("p (t e) -> p t e", e=ETL),
                xin_d.ap().rearrange("(t p) e -> p t e", p=P),
            )
            dbn = sbp.tile([DP, DC * ETL], F32, name="dbns")
            nc.sync.dma_start(
                dbn[:].rearrange("p (c e) -> p c e", e=ETL),
                dbm_d.ap().rearrange("(c p) e -> p c e", p=DP),
            )
            idn = sbp.tile([DP, DP], F32, name="idns")
            nc.sync.dma_start(idn[:], idn_d.ap())

            # ---------------- dictionary normalization ----------------
            # SE columns are the even echoes; view [p, c, e2, par][..., 0].
            dbn_se = dbn[:].rearrange("p (c e2 par) -> p c e2 par", par=2, e2=8)[
                :, :, :, 0
            ]
            dsq = sbp.tile([DP, DC * 8], F32, name="dsq")
            dsq3 = dsq[:].rearrange("p (c k) -> p c k", k=8)
            nc.scalar.activation(dsq3, dbn_se, Act.Square)
            dss = sbp.tile([DP, DC], F32, name="dss")
            nc.vector.tensor_reduce(dss[:], dsq3, axis=AX.X, op=Alu.add)
            dinv = _rsqrt_refined(nc, sbp, dss[:], name="dinv")
            dms8 = sbp.tile([DP, DC * 8], F32, name="dms8")
            nc.vector.tensor_mul(
                dms8[:].rearrange("p (c k) -> p c k", k=8),
                dbn_se,
                dinv[:].unsqueeze(2).broadcast_to([DP, DC, 8]),
            )

            # transpose [DP, 8] chunks -> dbT8 [8, 4000]
            dbT8 = sbp.tile([8, D], F32, name="dbT8")
            dms8_3 = dms8[:].rearrange("p (c k) -> p c k", k=8)
            for g in range(8):
                pt = psp.tile([P, HALF], F32, tag="enc", name=f"tp{g}")
                for j in range(4):
                    c = 4 * g + j
                    nc.tensor.transpose(
                        pt[0:8, j * DP : (j + 1) * DP],
                        dms8_3[:, c, :],
                        idn[:],
                    )
                nc.scalar.activation(
                    dbT8[:, g * 500 : (g + 1) * 500], pt[0:8, 0:500], Act.Copy
                )

            # ---------------- per-pixel signal sumsq ----------------
            xin_se = xin[:].rearrange("p (t e2 par) -> p t e2 par", par=2, e2=8)[
                :, :, :, 0
            ]
            xsq = sbp.tile([P, T * 8], F32, name="xsq")
            xsq3 = xsq[:].rearrange("p (t k) -> p t k", k=8)
            nc.scalar.activation(xsq3, xin_se, Act.Square)
            s2 = sbp.tile([P, T], F32, name="s2")
            nc.vector.tensor_reduce(s2[:], xsq3, axis=AX.X, op=Alu.add)
            rn = _rsqrt_refined(nc, sbp, s2[:], name="rn")

            # ---------------- main loop ----------------
            # Pairwise-max tree: pm1[d] = max over {d, d+2000}; pm2 over
            # {d, d+1000, ...}; pm3 (500 wide) over {d + 500j, j in 0..7}.
            # max8+max_index on pm3 give the exact max value and its pm3
            # position; the host resolves which of the 8 aliased atoms it was.
            m_all = sbp.tile([P, T], F32, name="m_all")
            gi_all = sbp.tile([P, T * 8], U32, name="gi_all")
            for t in range(T):
                lhsT = xT8[:, t * P : (t + 1) * P]
                psA = psp.tile([P, HALF], F32, tag="enc", name=f"psA{t}")
                psB = psp.tile([P, HALF], F32, tag="enc", name=f"psB{t}")
                for h, ps in enumerate((psA, psB)):
                    for off, w in CHUNKS:
                        nc.tensor.matmul(
                            ps[:, off : off + w],
                            lhsT,
                            dbT8[:, h * HALF + off : h * HALF + off + w],
                            start=True,
                            stop=True,
                        )
                sbB = esp.tile([P, HALF], F32, tag="sbB", name=f"sbB{t}")
                nc.scalar.activation(sbB[:], psB[:], Act.Copy)
                pm1 = scp.tile([P, HALF], F32, tag="pm1", name=f"pm1_{t}")
                nc.vector.tensor_max(pm1[:], psA[:], sbB[:])
                pm2 = scp.tile([P, HALF // 2], F32, tag="pm2", name=f"pm2_{t}")
                nc.vector.tensor_max(pm2[:], pm1[:, : HALF // 2], pm1[:, HALF // 2 :])
                pm3 = scp.tile([P, HALF // 4], F32, tag="pm3", name=f"pm3_{t}")
                nc.vector.tensor_max(
                    pm3[:], pm2[:, : HALF // 4], pm2[:, HALF // 4 :]
                )
                mx8 = mxp.tile([P, 8], F32, tag="mx8", name=f"mx{t}")
                nc.vector.max(mx8[:], pm3[:])
                nc.vector.max_index(gi_all[:, t * 8 : t * 8 + 8], mx8[:], pm3[:])
                nc.scalar.activation(m_all[:, t : t + 1], mx8[:, 0:1], Act.Copy)

            # ---------------- epilogue ----------------
            gidx = sbp.tile([P, T], U32, name="gidx")
            nc.vector.tensor_copy(
                gidx[:], gi_all[:].rearrange("p (t k) -> p t k", k=8)[:, :, 0]
            )
            q = sbp.tile([P, T], F32, name="q")
            nc.vector.tensor_mul(q[:], m_all[:], rn[:])
            v = sbp.tile([P, T], F32, name="v")
            nc.vector.tensor_scalar(v[:], q[:], -2.0, 2.0, op0=Alu.mult, op1=Alu.add)
            v2 = sbp.tile([P, T], F32, name="v2")
            nc.vector.tensor_scalar_max(v2[:], v[:], 1e-12)
            y0 = sbp.tile([P, T], F32, name="y0")
            nc.scalar.activation(y0[:], v2[:], Act.Sqrt)
            yr = sbp.tile([P, T], F32, name="yr")
            nc.vector.reciprocal(yr[:], y0[:])
            t3 = sbp.tile([P, T], F32, name="t3")
            nc.vector.tensor_mul(t3[:], v2[:], yr[:])
            t4 = sbp.tile([P, T], F32, name="t4")
            nc.vector.tensor_add(t4[:], y0[:], t3[:])
            md = sbp.tile([P, T], F32, name="mdt")
            nc.vector.tensor_scalar_mul(md[:], t4[:], 0.5)

            nc.sync.dma_start(oidx_d.ap(), gidx[:])
            nc.sync.dma_start(omd_d.ap(), md[:])
            nc.sync.dma_start(om_d.ap(), m_all[:])

    nc.compile()
    return nc


_CACHED = {}


def get_program(num_devices=NCORES):
    if num_devices not in _CACHED:
        _CACHED[num_devices] = build_program(num_devices)
    return _CACHED[num_devices]


def make_in_maps(slice_signal, db_mag):
    x = np.ascontiguousarray(np.asarray(slice_signal, np.float32).reshape(N, ETL))
    dbm = np.ascontiguousarray(np.asarray(db_mag, np.float32))
    idn = np.eye(DP, dtype=np.float32)
    in_maps = []
    for k in range(NCORES):
        sh = x[k * NPC : (k + 1) * NPC]
        in_maps.append(
            {
                "xT8": np.ascontiguousarray(sh.T[0::2]),
                "xin": sh,
                "dbm": dbm,
                "idn": idn,
            }
        )
    return in_maps


def _resolve_candidates(q3, m, slice_signal, db_mag):
    """q3 [N] in [0,500): position of the per-pixel max in the depth-3
    pairwise-max tree; the true atom is one of q3 + 500*j, j in 0..7.
    Resolve with the exact device max value m: compute the 8 candidate dot
    products on host and pick the lowest-index one matching m."""
    x = np.asarray(slice_signal, np.float32).reshape(N, ETL)[:, 0::2]  # [N, 8]
    db = np.asarray(db_mag, np.float32)
    dbm = db[:, 0::2].astype(np.float64)
    nrm = np.sqrt((dbm * dbm).sum(1))
    nrm[nrm == 0] = 1.0
    dbh = (dbm / nrm[:, None]).astype(np.float32)  # [D, 8] normalized SE cols
    cand = q3[:, None] + 500 * np.arange(8)[None, :]  # [N, 8]
    cd = np.einsum("nk,njk->nj", x, dbh[cand], optimize=True)  # [N, 8] fp32
    err = np.abs(cd - m[:, None])
    tol = np.maximum(np.abs(m), 1e-3)[:, None] * 1e-5
    ok = err <= tol
    # lowest candidate index among matches; fallback to argmin error
    first_ok = np.where(ok.any(1), ok.argmax(1), err.argmin(1))
    return cand[np.arange(N), first_ok]


def kernel(slice_signal, db_mag, db_t2s_s, db_b1s, delta_t_r2p_ms, _results_hook=None):
    # mask sanity: the kernel contracts over even echoes (the SE columns).
    mask = (np.asarray(delta_t_r2p_ms, np.float32) * 1e-3 < 1e-3)
    assert mask.tolist() == [True, False] * (ETL // 2), (
        "kernel hardcodes even-echo SE mask; got %s" % mask
    )
    nc = get_program(NCORES)
    in_maps = make_in_maps(slice_signal, db_mag)
    res = bass_utils.run_bass_kernel_spmd(
        nc, in_maps, core_ids=list(range(NCORES))
    )
    if _results_hook is not None:
        _results_hook(res)
    q3 = np.concatenate(
        [res.results[k]["oidx"].T.reshape(NPC) for k in range(NCORES)]
    ).astype(np.int64)
    md = np.concatenate([res.results[k]["omd"].T.reshape(NPC) for k in range(NCORES)])
    m = np.concatenate([res.results[k]["om"].T.reshape(NPC) for k in range(NCORES)])
    idx = _resolve_candidates(q3, m, slice_signal, db_mag)
    t2 = np.asarray(db_t2s_s, np.float32)[idx].reshape(NX, NY)
    b1 = np.asarray(db_b1s, np.float32)[idx].reshape(NX, NY)
    return t2, b1, md.astype(np.float32)
